# revision 45
# baseline (speedup 1.0000x reference)
"""Causal attention (B=8, N=4096 flattened 64x64, d=128) on 8 trn2 cores.

Sharding: data-parallel over batch -- core b gets batch element b.

Per-core algorithm (flash-style, transposed orientation):
  inputs per core (host pre-transposed):
    qT [128, 4096] bf16  (c on partitions, query pos on free)
    kT [128, 4096] bf16
    vT [128, 4096] bf16  (k-within-tile on partitions: vT[p, 128j+c] = v[128j+p, c])
  loop q-chunks of 512, k-tiles of 128 (j = 0..4t+3):
    S^T[k, q] = kT_j.T @ qT_chunk          (PE, PSUM, N=512, bf16 moving)
    E = exp(S^T / sqrt(128))  -> bf16      (ScalarE, PSUM->SBUF, groups of 3 j)
    causal mask on diagonal tiles          (GpSimd affine_select, fill 0)
    O^T += v_j.T @ E_j                     (PE, accumulate in PSUM over j)
    denom[q] += sum_k E_j[k, q]            (split: PE all-ones matmul / DVE adds)
  Diagonal k-tiles narrow their S/PV matmuls to the non-masked column range;
  the skipped PSUM prefix holds garbage, exp of it is zero-filled by the
  affine_select (select semantics, so inf/NaN get dropped, not multiplied).
  outputs per core: outT [128, 4096] (unnormalized O^T), den [1, 4096]
  host: out = (outT / den).T

Chunks are processed in order 1..7,0 so the tail (last exp -> PV -> copy ->
DMA) is the smallest chunk. Inputs arrive as 4 independent pieces per tensor
(separate SBUF tiles, so the piece DMAs don't serialize on WAW tracking),
spread across the scalar/vector/sync/gpsimd queues, widest pieces last.

No max-subtraction in softmax: scores are ~N(0,1) (max |s| < ~7), exp is safe
in fp32 and softmax is shift-invariant. Masked probabilities are exactly zero
(select with fill=0), matching the reference's `softmax(.)*allowed`.
"""

import math

import ml_dtypes
import numpy as np

import concourse.bacc as bacc
import concourse.mybir as mybir
import concourse.tile as tile
from concourse.bass import ts, ds
from concourse.bass_utils import run_bass_kernel_spmd

P = 128
NSEQ = 4096
QCH = 512              # query positions per chunk
NCH = NSEQ // QCH      # 8 chunks
GROUP = 3              # k-tiles per exp group (3 PSUM banks; x2 buffered)
SCALE = 1.0 / math.sqrt(128.0)
F32 = mybir.dt.float32
I32 = mybir.dt.int32
BF16 = mybir.dt.bfloat16
N_CORES = 8
PE_DEN_MOD = 2         # every PE_DEN_MOD groups -> denominator matmul on PE

# Schraudolph exp2 bit-trick on DVE for a few below-diagonal groups:
# e = bitcast_f32(int32(s*SCH_A + SCH_B)); max rel err ~3% on those
# softmax weights only (bounded contribution to the output), frees the
# ScalarE activation queue which is the critical engine.
SCH_A = SCALE * (1 << 23) / math.log(2.0)      # 1069693.74
SCH_B = float((127 << 23) - 366500)            # min-max-rel bias
DVE_EXP_GROUPS = set()  # net loss on HW: DVE queue serialization bubbles
GP_DEN_CHUNKS = 99      # gpsimd den adds: also a net loss, disabled

CHUNK_ORDER = [0, 2, 3, 4, 5, 6, 7, 1]   # start AND end on small chunks
# input pieces (column ranges); piece 0 comes packed in blk0.  q's
# [512:1024) slice (chunk 1, processed last) is fetched dead last.
KV_PIECES = [(0, 512), (512, 1536), (1536, 2560), (2560, 4096)]
Q_PIECES = [(0, 512), (512, 1024), (1024, 1536), (1536, 2560), (2560, 4096)]

_nc_cache = []


def _build():
    nc = bacc.Bacc("TRN2", target_bir_lowering=False, debug=False,
                   num_devices=N_CORES)
    qT = nc.dram_tensor("qT", [P, NSEQ], BF16, kind="ExternalInput").ap()
    kT = nc.dram_tensor("kT", [P, NSEQ], BF16, kind="ExternalInput").ap()
    vT = nc.dram_tensor("vT", [P, NSEQ], BF16, kind="ExternalInput").ap()
    # ramp-critical first block packed host-side as kT[:512] | qT[:512] |
    # qT[1024:1536] | vT[:512]: 4 KB HBM lines instead of 1 KB, so the
    # early 512 KB moves at ~2-3x the packet rate
    blk0 = nc.dram_tensor("blk0", [P, 4 * 512], BF16,
                          kind="ExternalInput").ap()
    outT = nc.dram_tensor("outT", [P, NSEQ], F32, kind="ExternalOutput").ap()
    den = nc.dram_tensor("den", [1, NSEQ], F32, kind="ExternalOutput").ap()

    exp_fn = mybir.ActivationFunctionType.Exp
    is_ge = mybir.AluOpType.is_ge

    with tile.TileContext(nc) as tc:
        with (
            tc.tile_pool(name="const", bufs=1) as cpool,
            tc.tile_pool(name="epool", bufs=13) as epool,
            tc.tile_pool(name="qpool", bufs=12) as qpool,
            tc.tile_pool(name="upool", bufs=2) as upool,
            tc.tile_pool(name="spool", bufs=2) as spool,
            tc.tile_pool(name="ps_s", bufs=2, space="PSUM") as ps_pool,
            tc.tile_pool(name="ps_o", bufs=1, space="PSUM") as po_pool,
            tc.tile_pool(name="ps_d", bufs=1, space="PSUM") as pd_pool,
        ):
            ones_sq = cpool.tile([P, P], BF16)
            nc.gpsimd.memset(ones_sq, 1.0)
            # pre-warm the PE during the input-DMA wait so the HAM clock
            # gate is at 2.4 GHz when real work starts; chunk order [0]'s
            # first denominator matmul clears the db bank anyway
            warm_db = pd_pool.tile([P, QCH], F32, tag="db", name="warm")
            for wi in range(16):
                nc.tensor.matmul(warm_db[:, ds(0, 64)], ones_sq,
                                 ones_sq[:, :64], start=True, stop=True)

            # input pieces: separate tiles so their DMAs are independent
            # (a single destination tile serializes the piece DMAs WAW).
            # Queue split keeps piece-0 triggers first on each queue.
            # two separate tiles so the first S matmuls (k0+q0) don't
            # falsely depend on the second half's DMA (deps are
            # tile-granular)
            blk0a_sb = cpool.tile([P, 1024], BF16, name="blk0a")
            blk0b_sb = cpool.tile([P, 1024], BF16, name="blk0b")
            kp, qp, vp = {}, {}, {}
            kp[0] = blk0a_sb[:, ds(0, 512)]
            qp[0] = blk0a_sb[:, ds(512, 512)]
            qp[2] = blk0b_sb[:, ds(0, 512)]
            vp[0] = blk0b_sb[:, ds(512, 512)]
            for pi, (c0, c1) in enumerate(KV_PIECES):
                if pi == 0:
                    continue
                kp[pi] = cpool.tile([P, c1 - c0], BF16, name=f"kp{pi}")
                vp[pi] = cpool.tile([P, c1 - c0], BF16, name=f"vp{pi}")
            for pi, (c0, c1) in enumerate(Q_PIECES):
                if pi in (0, 2):
                    continue
                qp[pi] = cpool.tile([P, c1 - c0], BF16, name=f"qp{pi}")
            # ring discipline: scalar ring carries k0|q0 (everything the
            # first S group needs), sync ring is FIFO [q2|v0, then pieces
            # in first-use order] — the DMA engines round-robin across
            # rings, so the first block owns the early bandwidth
            nc.scalar.dma_start(blk0a_sb, blk0[:, ds(0, 1024)])
            nc.sync.dma_start(blk0b_sb, blk0[:, ds(1024, 1024)])
            for tname, pi in (("k", 1), ("v", 1), ("q", 3),
                              ("k", 2), ("v", 2), ("q", 4),
                              ("k", 3), ("v", 3), ("q", 1)):
                tbl = Q_PIECES if tname == "q" else KV_PIECES
                c0, c1 = tbl[pi]
                dst, src = {"q": (qp, qT), "k": (kp, kT),
                            "v": (vp, vT)}[tname]
                nc.sync.dma_start(dst[pi], src[:, ds(c0, c1 - c0)])

            def piece_of(table, col):
                for pi, (c0, c1) in enumerate(table):
                    if c0 <= col < c1:
                        return pi, c0
                raise AssertionError(col)

            def k_tile(j):
                pi, c0 = piece_of(KV_PIECES, j * P)
                return kp[pi][:, ds(j * P - c0, P)]

            def v_tile(j):
                pi, c0 = piece_of(KV_PIECES, j * P)
                return vp[pi][:, ds(j * P - c0, P)]

            def q_chunk(t):
                pi, c0 = piece_of(Q_PIECES, t * QCH)
                return qp[pi][:, ds(t * QCH - c0, QCH)]

            o_tiles, db_tiles = {}, {}

            def emit_pv(ops):
                # deferred PV / den matmuls / chunk flushes for one group
                # (software pipelining: keeps the in-order PE queue's S
                # matmuls ahead of PVs that wait on the gpsimd select)
                for op in ops:
                    kind = op[0]
                    if kind == "pv":
                        _, t, pos, j, nj, e_sb, d = op
                        dd = j - 4 * t
                        off = max(dd, 0) * P
                        nc.tensor.matmul(
                            o_tiles[t][:, ds(off, QCH - off)],
                            v_tile(j),
                            e_sb[:, ds(d * QCH + off, QCH - off)],
                            start=(pos == 0), stop=(pos == nj - 1))
                    elif kind == "den":
                        _, t, den_blk, st_, sp_ = op
                        nc.tensor.matmul(db_tiles[t], ones_sq, den_blk,
                                         start=st_, stop=sp_)
                    else:   # flush: copy chunk outputs + DMA out
                        _, t = op
                        o_ps, db_ps = o_tiles[t], db_tiles[t]
                        out_sb = spool.tile([P, QCH], F32, tag="osb",
                                            name=f"osb{t}")
                        den_sb = spool.tile([1, QCH], F32, tag="den",
                                            name=f"den{t}")
                        if t == CHUNK_ORDER[-1]:   # tail: split engines
                            nc.scalar.copy(out_sb, o_ps)
                            nc.vector.tensor_copy(den_sb, db_ps[0:1, :])
                            nc.sync.dma_start(outT[:, ts(t, QCH)], out_sb)
                            nc.scalar.dma_start(den[:, ts(t, QCH)], den_sb)
                        else:
                            nc.vector.tensor_copy(out_sb, o_ps)
                            nc.vector.tensor_copy(den_sb, db_ps[0:1, :])
                            nc.sync.dma_start(outT[:, ts(t, QCH)], out_sb)
                            nc.sync.dma_start(den[:, ts(t, QCH)], den_sb)

            # global tile sequence: exp groups of GROUP tiles flow across
            # chunk boundaries so the activation stream never pauses at a
            # chunk edge.  The final chunk runs its diagonal tiles first
            # so the tail (last exp -> PV -> copy -> DMA) has no selects.
            entries = []
            for t in CHUNK_ORDER:
                nj = 4 * (t + 1)          # causal: k-tiles 0..4t+3
                seq = list(range(nj))
                if t == CHUNK_ORDER[-1]:
                    seq = seq[4 * t:] + seq[:4 * t]
                for pos, j in enumerate(seq):
                    entries.append((t, pos, j, nj))

            DEN_WIN = 2 * GROUP     # tiles per denominator matmul window
            den_state = {}          # t -> (prev_slice, qacc, count, n_mms)
            pv_pending = []
            groups = [entries[g0:g0 + GROUP]
                      for g0 in range(0, len(entries), GROUP)]
            for members in groups:
                gn = len(members)
                s_ps = ps_pool.tile([P, gn * QCH], F32, tag="s",
                                    padded_shape=[P, GROUP * QCH])
                for d, (t, pos, j, nj) in enumerate(members):
                    dd = j - 4 * t
                    off = max(dd, 0) * P   # fully-masked column prefix
                    nc.tensor.matmul(
                        s_ps[:, ds(d * QCH + off, QCH - off)],
                        k_tile(j), q_chunk(t)[:, ds(off, QCH - off)],
                        start=True, stop=True)
                e_sb = epool.tile([P, gn * QCH], BF16, tag="e",
                                  padded_shape=[P, GROUP * QCH])
                # if the group's first tile is diagonal, its masked
                # column prefix is at the window start -> skip it
                t0, _, j0_, _ = members[0]
                off0 = max(j0_ - 4 * t0, 0) * P
                nc.scalar.activation(e_sb[:, ds(off0, gn * QCH - off0)],
                                     s_ps[:, ds(off0, gn * QCH - off0)],
                                     exp_fn, scale=SCALE)

                # causal mask on diagonal tiles (j in [4t, 4t+4)):
                # keep where qcol - k - 128*dd >= 0, else fill 0.
                # Covers the skipped S prefix too (exp of stale PSUM).
                for d, (t, pos, j, nj) in enumerate(members):
                    dd = j - 4 * t
                    if dd >= 0:
                        w = (dd + 1) * P
                        reg = e_sb[:, ds(d * QCH, w)]
                        nc.gpsimd.affine_select(
                            out=reg, in_=reg, compare_op=is_ge,
                            fill=0.0, base=-dd * P, pattern=[[1, w]],
                            channel_multiplier=-1)

                # build this group's deferred op list; run the per-chunk
                # denominator state machine (DVE bf16 adds now, ones
                # matmuls into db deferred with the PVs)
                ops = []
                for d, (t, pos, j, nj) in enumerate(members):
                    if pos == 0:
                        o_tiles[t] = po_pool.tile([P, QCH], F32, tag="o",
                                                  name=f"o{t}")
                        db_tiles[t] = pd_pool.tile([P, QCH], F32, tag="db",
                                                   name=f"db{t}")
                        den_state[t] = [None, None, 0, 0]
                    ops.append(("pv", t, pos, j, nj, e_sb, d))
                    st = den_state[t]
                    sl = e_sb[:, ts(d, QCH)]
                    if st[1] is None:
                        if st[0] is None:
                            st[0] = sl
                        else:
                            qacc = qpool.tile([P, QCH], BF16, tag="qacc")
                            nc.vector.tensor_add(qacc, st[0], sl)
                            st[0], st[1], st[2] = None, qacc, 2
                    else:
                        nc.vector.tensor_add(st[1], st[1], sl)
                        st[2] += 1
                    last_of_chunk = pos == nj - 1
                    if st[2] >= DEN_WIN or last_of_chunk:
                        if st[1] is not None:
                            den_blk = st[1]
                        else:
                            den_blk = st[0]   # lone raw slice
                        if den_blk is not None:
                            ops.append(("den", t, den_blk,
                                        st[3] == 0, last_of_chunk))
                            st[3] += 1
                        st[0], st[1], st[2] = None, None, 0
                    if last_of_chunk:
                        ops.append(("flush", t))

                if len(pv_pending) >= 2:
                    emit_pv(pv_pending.pop(0))
                pv_pending.append(ops)

            for ops in pv_pending:
                emit_pv(ops)

    nc.compile()
    return nc


def _get_nc():
    if not _nc_cache:
        _nc_cache.append(_build())
    return _nc_cache[0]


def _prep(query, key, value):
    B, H, W, C = query.shape
    CV = value.shape[-1]
    n = H * W
    q = (np.asarray(query, np.float32).reshape(B, n, C).transpose(0, 2, 1)
         .astype(ml_dtypes.bfloat16))
    q = np.ascontiguousarray(q)
    k = np.ascontiguousarray(
        np.asarray(key, np.float32).reshape(B, n, C).transpose(0, 2, 1)
        .astype(ml_dtypes.bfloat16))
    # vT[b, p, 128j+c] = v[b, 128j+p, c]: k-within-tile on partitions, so a
    # [128, 128] SBUF slice is directly the PV weight tile, and the HBM
    # lines are long and contiguous (8 KB per partition row)
    v = (np.asarray(value, np.float32).reshape(B, n // P, P, CV)
         .transpose(0, 2, 1, 3).reshape(B, P, n // P * CV)
         .astype(ml_dtypes.bfloat16))
    v = np.ascontiguousarray(v)
    b0 = np.ascontiguousarray(
        np.concatenate([k[:, :, :512], q[:, :, :512], q[:, :, 1024:1536],
                        v[:, :, :512]], axis=2))
    return q, k, v, b0


def kernel(query, key, value):
    B, H, W, C = query.shape
    CV = value.shape[-1]
    n = H * W
    q, k, v, b0 = _prep(query, key, value)

    nc = _get_nc()
    in_maps = [{"qT": q[b], "kT": k[b], "vT": v[b], "blk0": b0[b]}
               for b in range(B)]
    res = run_bass_kernel_spmd(nc, in_maps, core_ids=list(range(N_CORES)))

    out = np.empty((B, n, CV), np.float32)
    for b in range(B):
        oT = res.results[b]["outT"]          # [128, 4096] unnormalized O^T
        dn = res.results[b]["den"]           # [1, 4096]
        out[b] = (oT / dn).T
    return out.reshape(B, H, W, CV)


# revision 48
# speedup vs baseline: 1.0224x; 1.0224x over previous
"""Causal attention (B=8, N=4096 flattened 64x64, d=128) on 8 trn2 cores.

Sharding: data-parallel over batch -- core b gets batch element b.

Per-core algorithm (flash-style, transposed orientation):
  inputs per core (host pre-transposed):
    qT [128, 4096] bf16  (c on partitions, query pos on free)
    kT [128, 4096] bf16
    vT [128, 4096] bf16  (k-within-tile on partitions: vT[p, 128j+c] = v[128j+p, c])
  loop q-chunks of 512, k-tiles of 128 (j = 0..4t+3):
    S^T[k, q] = kT_j.T @ qT_chunk          (PE, PSUM, N=512, bf16 moving)
    E = exp(S^T / sqrt(128))  -> bf16      (ScalarE, PSUM->SBUF, groups of 3 j)
    causal mask on diagonal tiles          (GpSimd affine_select, fill 0)
    O^T += v_j.T @ E_j                     (PE, accumulate in PSUM over j)
    denom[q] += sum_k E_j[k, q]            (split: PE all-ones matmul / DVE adds)
  Diagonal k-tiles narrow their S/PV matmuls to the non-masked column range;
  the skipped PSUM prefix holds garbage, exp of it is zero-filled by the
  affine_select (select semantics, so inf/NaN get dropped, not multiplied).
  outputs per core: outT [128, 4096] (unnormalized O^T), den [1, 4096]
  host: out = (outT / den).T

Chunks are processed in order 1..7,0 so the tail (last exp -> PV -> copy ->
DMA) is the smallest chunk. Inputs arrive as 4 independent pieces per tensor
(separate SBUF tiles, so the piece DMAs don't serialize on WAW tracking),
spread across the scalar/vector/sync/gpsimd queues, widest pieces last.

No max-subtraction in softmax: scores are ~N(0,1) (max |s| < ~7), exp is safe
in fp32 and softmax is shift-invariant. Masked probabilities are exactly zero
(select with fill=0), matching the reference's `softmax(.)*allowed`.
"""

import math

import ml_dtypes
import numpy as np

import concourse.bacc as bacc
import concourse.mybir as mybir
import concourse.tile as tile
from concourse.bass import ts, ds
from concourse.bass_utils import run_bass_kernel_spmd

P = 128
NSEQ = 4096
QCH = 512              # query positions per chunk
NCH = NSEQ // QCH      # 8 chunks
GROUP = 3              # k-tiles per exp group (3 PSUM banks; x2 buffered)
SCALE = 1.0 / math.sqrt(128.0)
F32 = mybir.dt.float32
I32 = mybir.dt.int32
BF16 = mybir.dt.bfloat16
N_CORES = 8
PE_DEN_MOD = 2         # every PE_DEN_MOD groups -> denominator matmul on PE

# Schraudolph exp2 bit-trick on DVE for a few below-diagonal groups:
# e = bitcast_f32(int32(s*SCH_A + SCH_B)); max rel err ~3% on those
# softmax weights only (bounded contribution to the output), frees the
# ScalarE activation queue which is the critical engine.
SCH_A = SCALE * (1 << 23) / math.log(2.0)      # 1069693.74
SCH_B = float((127 << 23) - 366500)            # min-max-rel bias
DVE_EXP_GROUPS = set()  # net loss on HW: DVE queue serialization bubbles
GP_DEN_CHUNKS = 99      # gpsimd den adds: also a net loss, disabled

CHUNK_ORDER = [0, 2, 3, 4, 5, 6, 7, 1]   # start AND end on small chunks
# input pieces (column ranges); piece 0 comes packed in blk0.  q's
# [512:1024) slice (chunk 1, processed last) is fetched dead last.
KV_PIECES = [(0, 512), (512, 1536), (1536, 2560), (2560, 4096)]
Q_PIECES = [(0, 512), (512, 1024), (1024, 1536), (1536, 2560), (2560, 4096)]

_nc_cache = []


def _build():
    nc = bacc.Bacc("TRN2", target_bir_lowering=False, debug=False,
                   num_devices=N_CORES)
    qT = nc.dram_tensor("qT", [P, NSEQ], BF16, kind="ExternalInput").ap()
    kT = nc.dram_tensor("kT", [P, NSEQ], BF16, kind="ExternalInput").ap()
    vT = nc.dram_tensor("vT", [P, NSEQ], BF16, kind="ExternalInput").ap()
    # ramp-critical first block packed host-side as kT[:512] | qT[:512] |
    # qT[1024:1536] | vT[:512]: 4 KB HBM lines instead of 1 KB, so the
    # early 512 KB moves at ~2-3x the packet rate
    blk0 = nc.dram_tensor("blk0", [P, 4 * 512], BF16,
                          kind="ExternalInput").ap()
    outT = nc.dram_tensor("outT", [P, NSEQ], F32, kind="ExternalOutput").ap()
    den = nc.dram_tensor("den", [1, NSEQ], F32, kind="ExternalOutput").ap()

    exp_fn = mybir.ActivationFunctionType.Exp
    is_ge = mybir.AluOpType.is_ge

    with tile.TileContext(nc) as tc:
        with (
            tc.tile_pool(name="const", bufs=1) as cpool,
            tc.tile_pool(name="epool", bufs=13) as epool,
            tc.tile_pool(name="qpool", bufs=12) as qpool,
            tc.tile_pool(name="upool", bufs=2) as upool,
            tc.tile_pool(name="spool", bufs=2) as spool,
            tc.tile_pool(name="ps_s", bufs=2, space="PSUM") as ps_pool,
            tc.tile_pool(name="ps_o", bufs=1, space="PSUM") as po_pool,
            tc.tile_pool(name="ps_d", bufs=1, space="PSUM") as pd_pool,
        ):
            ones_sq = cpool.tile([P, P], BF16)
            nc.gpsimd.memset(ones_sq, 1.0)
            # pre-warm the PE during the input-DMA wait so the HAM clock
            # gate is at 2.4 GHz when real work starts; chunk order [0]'s
            # first denominator matmul clears the db bank anyway
            # enough warmup matmuls to bridge the input-DMA wait (~4us):
            # a PE-idle gap > ~3.4us would re-throttle the HAM clock gate
            # and the first real matmul groups would run at 1.2 GHz
            warm_db = pd_pool.tile([P, QCH], F32, tag="db", name="warm")
            for wi in range(72):
                nc.tensor.matmul(warm_db[:, ds(0, 64)], ones_sq,
                                 ones_sq[:, :64], start=True, stop=True)

            # input pieces: separate tiles so their DMAs are independent
            # (a single destination tile serializes the piece DMAs WAW).
            # Queue split keeps piece-0 triggers first on each queue.
            blk0_sb = cpool.tile([P, 4 * 512], BF16, name="blk0")
            kp, qp, vp = {}, {}, {}
            kp[0] = blk0_sb[:, ds(0, 512)]
            qp[0] = blk0_sb[:, ds(512, 512)]
            qp[2] = blk0_sb[:, ds(1024, 512)]
            vp[0] = blk0_sb[:, ds(1536, 512)]
            for pi, (c0, c1) in enumerate(KV_PIECES):
                if pi == 0:
                    continue
                kp[pi] = cpool.tile([P, c1 - c0], BF16, name=f"kp{pi}")
                vp[pi] = cpool.tile([P, c1 - c0], BF16, name=f"vp{pi}")
            for pi, (c0, c1) in enumerate(Q_PIECES):
                if pi in (0, 2):
                    continue
                qp[pi] = cpool.tile([P, c1 - c0], BF16, name=f"qp{pi}")
            # ring discipline: scalar ring carries blk0's first 768 cols
            # (k0 + half of q0), sync ring is FIFO [rest of blk0, then
            # pieces in first-use order] — the DMA engines round-robin
            # across rings, so blk0 owns the early bandwidth
            nc.scalar.dma_start(blk0_sb[:, ds(0, 768)], blk0[:, ds(0, 768)])
            nc.sync.dma_start(blk0_sb[:, ds(768, 1280)],
                              blk0[:, ds(768, 1280)])
            for tname, pi in (("k", 1), ("v", 1), ("q", 3),
                              ("k", 2), ("v", 2), ("q", 4),
                              ("k", 3), ("v", 3), ("q", 1)):
                tbl = Q_PIECES if tname == "q" else KV_PIECES
                c0, c1 = tbl[pi]
                dst, src = {"q": (qp, qT), "k": (kp, kT),
                            "v": (vp, vT)}[tname]
                nc.sync.dma_start(dst[pi], src[:, ds(c0, c1 - c0)])

            def piece_of(table, col):
                for pi, (c0, c1) in enumerate(table):
                    if c0 <= col < c1:
                        return pi, c0
                raise AssertionError(col)

            def k_tile(j):
                pi, c0 = piece_of(KV_PIECES, j * P)
                return kp[pi][:, ds(j * P - c0, P)]

            def v_tile(j):
                pi, c0 = piece_of(KV_PIECES, j * P)
                return vp[pi][:, ds(j * P - c0, P)]

            def q_chunk(t):
                pi, c0 = piece_of(Q_PIECES, t * QCH)
                return qp[pi][:, ds(t * QCH - c0, QCH)]

            o_tiles, db_tiles = {}, {}

            def emit_pv(ops):
                # deferred PV / den matmuls / chunk flushes for one group
                # (software pipelining: keeps the in-order PE queue's S
                # matmuls ahead of PVs that wait on the gpsimd select)
                for op in ops:
                    kind = op[0]
                    if kind == "pv":
                        _, t, pos, j, nj, e_sb, d = op
                        dd = j - 4 * t
                        off = max(dd, 0) * P
                        nc.tensor.matmul(
                            o_tiles[t][:, ds(off, QCH - off)],
                            v_tile(j),
                            e_sb[:, ds(d * QCH + off, QCH - off)],
                            start=(pos == 0), stop=(pos == nj - 1))
                    elif kind == "den":
                        _, t, den_blk, st_, sp_ = op
                        nc.tensor.matmul(db_tiles[t], ones_sq, den_blk,
                                         start=st_, stop=sp_)
                    else:   # flush: copy chunk outputs + DMA out
                        _, t = op
                        o_ps, db_ps = o_tiles[t], db_tiles[t]
                        out_sb = spool.tile([P, QCH], F32, tag="osb",
                                            name=f"osb{t}")
                        den_sb = spool.tile([1, QCH], F32, tag="den",
                                            name=f"den{t}")
                        if t == CHUNK_ORDER[-1]:   # tail: split engines
                            nc.scalar.copy(out_sb, o_ps)
                            nc.vector.tensor_copy(den_sb, db_ps[0:1, :])
                            nc.sync.dma_start(outT[:, ts(t, QCH)], out_sb)
                            nc.scalar.dma_start(den[:, ts(t, QCH)], den_sb)
                        else:
                            nc.vector.tensor_copy(out_sb, o_ps)
                            nc.vector.tensor_copy(den_sb, db_ps[0:1, :])
                            nc.sync.dma_start(outT[:, ts(t, QCH)], out_sb)
                            nc.sync.dma_start(den[:, ts(t, QCH)], den_sb)

            # global tile sequence: exp groups of GROUP tiles flow across
            # chunk boundaries so the activation stream never pauses at a
            # chunk edge.  The final chunk runs its diagonal tiles first
            # so the tail (last exp -> PV -> copy -> DMA) has no selects.
            entries = []
            for t in CHUNK_ORDER:
                nj = 4 * (t + 1)          # causal: k-tiles 0..4t+3
                seq = list(range(nj))
                if t == CHUNK_ORDER[-1]:
                    seq = seq[4 * t:] + seq[:4 * t]
                for pos, j in enumerate(seq):
                    entries.append((t, pos, j, nj))

            DEN_WIN = 2 * GROUP     # tiles per denominator matmul window
            den_state = {}          # t -> (prev_slice, qacc, count, n_mms)
            pv_pending = []
            groups = [entries[g0:g0 + GROUP]
                      for g0 in range(0, len(entries), GROUP)]
            for members in groups:
                gn = len(members)
                s_ps = ps_pool.tile([P, gn * QCH], F32, tag="s",
                                    padded_shape=[P, GROUP * QCH])
                for d, (t, pos, j, nj) in enumerate(members):
                    dd = j - 4 * t
                    off = max(dd, 0) * P   # fully-masked column prefix
                    nc.tensor.matmul(
                        s_ps[:, ds(d * QCH + off, QCH - off)],
                        k_tile(j), q_chunk(t)[:, ds(off, QCH - off)],
                        start=True, stop=True)
                e_sb = epool.tile([P, gn * QCH], BF16, tag="e",
                                  padded_shape=[P, GROUP * QCH])
                # if the group's first tile is diagonal, its masked
                # column prefix is at the window start -> skip it
                t0, _, j0_, _ = members[0]
                off0 = max(j0_ - 4 * t0, 0) * P
                nc.scalar.activation(e_sb[:, ds(off0, gn * QCH - off0)],
                                     s_ps[:, ds(off0, gn * QCH - off0)],
                                     exp_fn, scale=SCALE)

                # causal mask on diagonal tiles (j in [4t, 4t+4)):
                # keep where qcol - k - 128*dd >= 0, else fill 0.
                # Covers the skipped S prefix too (exp of stale PSUM).
                for d, (t, pos, j, nj) in enumerate(members):
                    dd = j - 4 * t
                    if dd >= 0:
                        w = (dd + 1) * P
                        reg = e_sb[:, ds(d * QCH, w)]
                        nc.gpsimd.affine_select(
                            out=reg, in_=reg, compare_op=is_ge,
                            fill=0.0, base=-dd * P, pattern=[[1, w]],
                            channel_multiplier=-1)

                # build this group's deferred op list; run the per-chunk
                # denominator state machine (DVE bf16 adds now, ones
                # matmuls into db deferred with the PVs)
                ops = []
                for d, (t, pos, j, nj) in enumerate(members):
                    if pos == 0:
                        o_tiles[t] = po_pool.tile([P, QCH], F32, tag="o",
                                                  name=f"o{t}")
                        db_tiles[t] = pd_pool.tile([P, QCH], F32, tag="db",
                                                   name=f"db{t}")
                        den_state[t] = [None, None, 0, 0]
                    ops.append(("pv", t, pos, j, nj, e_sb, d))
                    st = den_state[t]
                    sl = e_sb[:, ts(d, QCH)]
                    if st[1] is None:
                        if st[0] is None:
                            st[0] = sl
                        else:
                            qacc = qpool.tile([P, QCH], BF16, tag="qacc")
                            nc.vector.tensor_add(qacc, st[0], sl)
                            st[0], st[1], st[2] = None, qacc, 2
                    else:
                        nc.vector.tensor_add(st[1], st[1], sl)
                        st[2] += 1
                    last_of_chunk = pos == nj - 1
                    if st[2] >= DEN_WIN or last_of_chunk:
                        if st[1] is not None:
                            den_blk = st[1]
                        else:
                            den_blk = st[0]   # lone raw slice
                        if den_blk is not None:
                            ops.append(("den", t, den_blk,
                                        st[3] == 0, last_of_chunk))
                            st[3] += 1
                        st[0], st[1], st[2] = None, None, 0
                    if last_of_chunk:
                        ops.append(("flush", t))

                if len(pv_pending) >= 2:
                    emit_pv(pv_pending.pop(0))
                pv_pending.append(ops)

            for ops in pv_pending:
                emit_pv(ops)

    nc.compile()
    return nc


def _get_nc():
    if not _nc_cache:
        _nc_cache.append(_build())
    return _nc_cache[0]


def _prep(query, key, value):
    B, H, W, C = query.shape
    CV = value.shape[-1]
    n = H * W
    q = (np.asarray(query, np.float32).reshape(B, n, C).transpose(0, 2, 1)
         .astype(ml_dtypes.bfloat16))
    q = np.ascontiguousarray(q)
    k = np.ascontiguousarray(
        np.asarray(key, np.float32).reshape(B, n, C).transpose(0, 2, 1)
        .astype(ml_dtypes.bfloat16))
    # vT[b, p, 128j+c] = v[b, 128j+p, c]: k-within-tile on partitions, so a
    # [128, 128] SBUF slice is directly the PV weight tile, and the HBM
    # lines are long and contiguous (8 KB per partition row)
    v = (np.asarray(value, np.float32).reshape(B, n // P, P, CV)
         .transpose(0, 2, 1, 3).reshape(B, P, n // P * CV)
         .astype(ml_dtypes.bfloat16))
    v = np.ascontiguousarray(v)
    b0 = np.ascontiguousarray(
        np.concatenate([k[:, :, :512], q[:, :, :512], q[:, :, 1024:1536],
                        v[:, :, :512]], axis=2))
    return q, k, v, b0


def kernel(query, key, value):
    B, H, W, C = query.shape
    CV = value.shape[-1]
    n = H * W
    q, k, v, b0 = _prep(query, key, value)

    nc = _get_nc()
    in_maps = [{"qT": q[b], "kT": k[b], "vT": v[b], "blk0": b0[b]}
               for b in range(B)]
    res = run_bass_kernel_spmd(nc, in_maps, core_ids=list(range(N_CORES)))

    out = np.empty((B, n, CV), np.float32)
    for b in range(B):
        oT = res.results[b]["outT"]          # [128, 4096] unnormalized O^T
        dn = res.results[b]["den"]           # [1, 4096]
        out[b] = (oT / dn).T
    return out.reshape(B, H, W, CV)
